# revision 1
# baseline (speedup 1.0000x reference)
"""GCN layer (BGRL-style) on 8 Trainium2 NeuronCores.

Math: the reference computes
  log_softmax(relu((A_hat @ (X*norm_src)) @ W_conv * norm_dst + b) @ W2 + b2).
Aggregation is linear, so we aggregate raw normalized features and apply
W_conv after — no cross-core exchange of hidden states is needed.

Sharding: destination nodes are split into 8 contiguous blocks of 6250; each
core owns the edges whose dst falls in its block (partitioned on host).
Per 128-dst block the core, fully on-chip in one pass:
  - gathers feat[src] rows (bf16, 512 feats = 1KB) from a replicated DRAM
    table into one per-block SBUF tile with per-partition-scalar indirect
    DMAs ([128,1] int32 offsets, round-robined over 4 SWDGE queues — the only
    dynamic-DMA shape this runtime supports),
  - segment-sums TRANSPOSED via one-hot S matmuls (gathered f-chunks as
    lhsT, S as rhs) -> aggT [f x d] in PSUM, so the whole downstream chain
    needs no transposes: W_conv matmuls (weights as lhsT), a free-dim
    broadcast multiply by norm_dst, relu+bias (per-partition, h on
    partitions), W2 matmuls, bias outer-products via K=1 matmuls, and
    log_softmax, streaming fp32 logits out per block.
All 8 cores run one SPMD program, so the edge partition is padded to a
uniform tiles-per-block count T_B (pad lanes get sentinel dst 255 -> S row 0).
"""

import numpy as np

N = 50000
F = 512
H = 256
C = 64
P = 8
NPC = N // P            # 6250 dst nodes per core
NB = (NPC + 127) // 128  # 49 dst blocks per core (last block has 106 rows)
LAST = NPC - (NB - 1) * 128  # rows in last block
NQ = 4                  # SWDGE queues for indirect gathers

_cache = {}


def _indirect_gather(nc, mybir, out, table, off_ap, queue):
    """indirect_dma_start clone with a queue override (offsets: [128,1] i32)."""
    eng = nc.gpsimd
    out_ap = eng.lower_ap_dma(out, for_indirect_dma=True)
    in_ap = eng.lower_ap_dma(table, for_indirect_dma=True)
    offset_ap = eng.lower_ap_dma(off_ap)[0]
    in_ap.append(offset_ap)
    dyn = mybir.DynamicAccessPatternInfo(
        c=0,
        actual_ap=out.ap,
        indirect_dim_max_index=table.shape[0],
        offset_expr=[mybir.DynamicAccessPatternOffsetExpr(
            coef=table.shape[1],
            aff_expr=mybir.DynamicAccessPatternOffsetExprAffExpr(
                kind="IndirectArgId", arg_id=1),
        )],
    )
    in_ap[0].dynamic_ap_info = dyn
    return eng.add_instruction(mybir.InstDMACopy(
        name=nc.get_next_instruction_name(),
        queue=queue, mode="Copy", ins=in_ap, outs=out_ap,
        oob_is_err=True, cce_op=mybir.AluOpType.bypass,
    ))


def _build_program(T_B, bench_R=0, mode="full"):
    import concourse.mybir as mybir
    import concourse.tile as tile
    from concourse import bacc

    dt = mybir.dt
    NT = NB * T_B           # edge tiles per core

    nc = bacc.Bacc("TRN2", target_bir_lowering=False, debug=False,
                   num_devices=P, num_swdge_queues=NQ)

    feat_d = nc.dram_tensor("feat", [N, F], dt.bfloat16, kind="ExternalInput")
    idx_d = nc.dram_tensor("idx32", [128, NT], dt.int32, kind="ExternalInput")
    dstloc_d = nc.dram_tensor("dstloc", [128, NT], dt.bfloat16, kind="ExternalInput")
    # normdst materialized across partitions (DVE lanes cannot broadcast
    # along the partition dim): every row identical
    normdst_d = nc.dram_tensor("normdst", [128, NB * 128], dt.float32,
                               kind="ExternalInput")
    iota_d = nc.dram_tensor("iota", [128, 128], dt.bfloat16, kind="ExternalInput")
    wconv_d = nc.dram_tensor("wconv", [128, 4 * H], dt.bfloat16, kind="ExternalInput")
    w2_d = nc.dram_tensor("w2", [128, 2 * C], dt.bfloat16, kind="ExternalInput")
    ones_d = nc.dram_tensor("ones1", [1, 128], dt.bfloat16, kind="ExternalInput")
    # bconv as [128, 2] (h on partitions, one col per h-half)
    bconv_d = nc.dram_tensor("bconv", [128, 2], dt.float32, kind="ExternalInput")
    b2_d = nc.dram_tensor("b2r", [1, C], dt.bfloat16, kind="ExternalInput")
    out_d = nc.dram_tensor("out", [NPC, C], dt.float32, kind="ExternalOutput")

    qnames = ["qPoolDynamic"] + [f"qPoolDynamic{i}" for i in range(1, NQ)]

    with tile.TileContext(nc) as tc:
        with (
            tc.tile_pool(name="const", bufs=1) as cpool,
            tc.tile_pool(name="work", bufs=3) as wpool,
            tc.tile_pool(name="gath", bufs=3) as gpool,
            tc.tile_pool(name="psum", bufs=3, space="PSUM") as ppool,
            tc.tile_pool(name="psum1", bufs=2, space="PSUM") as ppool1,
        ):
            # --- constants / metadata, loaded once ---
            iota_t = cpool.tile([128, 128], dt.bfloat16, tag="iota")
            nc.sync.dma_start(iota_t[:], iota_d[:])
            wconv_t = cpool.tile([128, 4 * H], dt.bfloat16, tag="wconv")
            nc.sync.dma_start(wconv_t[:], wconv_d[:])
            w2_t = cpool.tile([128, 2 * C], dt.bfloat16, tag="w2")
            nc.sync.dma_start(w2_t[:], w2_d[:])
            ones_t = cpool.tile([1, 128], dt.bfloat16, tag="ones")
            nc.sync.dma_start(ones_t[:], ones_d[:])
            bconv_t = cpool.tile([128, 2], dt.float32, tag="bconv")
            nc.sync.dma_start(bconv_t[:], bconv_d[:])
            b2_t = cpool.tile([1, C], dt.bfloat16, tag="b2")
            nc.sync.dma_start(b2_t[:], b2_d[:])
            idx_t = cpool.tile([128, NT], dt.int32, tag="idx")
            nc.sync.dma_start(idx_t[:], idx_d[:])
            dstloc_t = cpool.tile([128, NT], dt.bfloat16, tag="dstloc")
            nc.sync.dma_start(dstloc_t[:], dstloc_d[:])
            normdst_t = cpool.tile([128, NB * 128], dt.float32, tag="normdst")
            nc.sync.dma_start(normdst_t[:], normdst_d[:])

            iota_rep = iota_t[:].rearrange("p (o n) -> p o n", o=1).broadcast_to(
                [128, T_B, 128]
            )

            def body():
                for b in range(NB):
                    # S one-hot: S[p, t*128+j] = (dstloc[p, b*T_B+t] == j)
                    S = wpool.tile([128, T_B, 128], dt.bfloat16, tag="S",
                                   bufs=2)
                    nc.vector.tensor_tensor(
                        S[:],
                        iota_rep,
                        dstloc_t[:, b * T_B:(b + 1) * T_B].broadcast_to(
                            [128, T_B, 128]
                        ),
                        op=mybir.AluOpType.is_equal,
                    )
                    g = gpool.tile([128, T_B, F], dt.bfloat16, tag="g")
                    for t in range(T_B):
                        ti = b * T_B + t
                        _indirect_gather(nc, mybir, g[:, t, :], feat_d[:],
                                         idx_t[:, ti:ti + 1], qnames[ti % NQ])
                    # aggT[k] [128f x 128d] accumulated in one PSUM bank
                    aggT = ppool.tile([128, F], dt.float32, tag="aggT")
                    for t in range(T_B):
                        for k in range(4):
                            nc.tensor.matmul(
                                aggT[:, k * 128:(k + 1) * 128],
                                g[:, t, k * 128:(k + 1) * 128],
                                S[:, t, :],
                                start=(t == 0), stop=(t == T_B - 1),
                            )
                    aggFT = wpool.tile([128, F], dt.bfloat16, tag="aggFT")
                    nc.scalar.activation(
                        aggFT[:], aggT[:], mybir.ActivationFunctionType.Copy,
                    )
                    # xT[half] [128h x 128d] = sum_k wconv[k,half].T @ aggFT[k]
                    xtp = ppool1.tile([128, H], dt.float32, tag="xtp")
                    for half in range(2):
                        for k in range(4):
                            nc.tensor.matmul(
                                xtp[:, half * 128:(half + 1) * 128],
                                wconv_t[:, k * H + half * 128:
                                        k * H + (half + 1) * 128],
                                aggFT[:, k * 128:(k + 1) * 128],
                                start=(k == 0), stop=(k == 3),
                            )
                    # x = relu(xT * norm_dst[d] + b_conv[h]); norm_dst along
                    # free dim (d), bias per partition (h)
                    xn = wpool.tile([128, H], dt.float32, tag="xn")
                    nc.vector.tensor_tensor(
                        xn[:].rearrange("p (o n) -> p o n", o=2),
                        xtp[:].rearrange("p (o n) -> p o n", o=2),
                        normdst_t[:, b * 128:(b + 1) * 128]
                        .rearrange("p (o n) -> p o n", o=1)
                        .broadcast_to([128, 2, 128]),
                        op=mybir.AluOpType.mult,
                    )
                    xts = wpool.tile([128, H], dt.bfloat16, tag="xts")
                    for half in range(2):
                        nc.scalar.activation(
                            xts[:, half * 128:(half + 1) * 128],
                            xn[:, half * 128:(half + 1) * 128],
                            mybir.ActivationFunctionType.Relu,
                            bias=bconv_t[:, half:half + 1],
                        )
                    # logits [128d x 64] = sum_half xts[half].T @ w2[half] + b2
                    lps = ppool1.tile([128, C], dt.float32, tag="lps")
                    for half in range(2):
                        nc.tensor.matmul(
                            lps[:], xts[:, half * 128:(half + 1) * 128],
                            w2_t[:, half * C:(half + 1) * C],
                            start=(half == 0), stop=False,
                        )
                    nc.tensor.matmul(lps[:], ones_t[:], b2_t[:],
                                     start=False, stop=True)
                    # log_softmax along classes
                    mneg = wpool.tile([128, 1], dt.float32, tag="mneg")
                    nc.vector.reduce_max(mneg[:], lps[:],
                                         axis=mybir.AxisListType.X, negate=True)
                    esc = wpool.tile([128, C], dt.float32, tag="esc")
                    ssum = wpool.tile([128, 1], dt.float32, tag="ssum")
                    nc.scalar.activation(
                        esc[:], lps[:], mybir.ActivationFunctionType.Exp,
                        bias=mneg[:], accum_out=ssum[:],
                    )
                    lse = wpool.tile([128, 1], dt.float32, tag="lse")
                    nc.scalar.activation(lse[:], ssum[:],
                                         mybir.ActivationFunctionType.Ln)
                    shift = wpool.tile([128, 1], dt.float32, tag="shift")
                    nc.vector.tensor_tensor(shift[:], mneg[:], lse[:],
                                            op=mybir.AluOpType.subtract)
                    osb = wpool.tile([128, C], dt.float32, tag="osb")
                    nc.vector.tensor_scalar_add(osb[:], lps[:], shift[:])
                    rows = 128 if b < NB - 1 else LAST
                    nc.sync.dma_start(out_d[b * 128:b * 128 + rows, :],
                                      osb[:rows, :])

            if bench_R:
                with tc.For_i(0, bench_R, 1):
                    body()
            else:
                body()

    nc.compile()
    return nc


def _prep(features, W_conv, b_conv, W2, b2, src, dst):
    import ml_dtypes
    bf16 = ml_dtypes.bfloat16

    E = src.shape[0]
    src = np.asarray(src).astype(np.int64)
    dst = np.asarray(dst).astype(np.int64)
    deg_out = np.bincount(src, minlength=N).astype(np.float32)
    deg_in = np.bincount(dst, minlength=N).astype(np.float32)
    norm_src = 1.0 / np.sqrt(deg_out)
    norm_dst = 1.0 / np.sqrt(deg_in)

    feat_n = (np.asarray(features, np.float32) * norm_src[:, None]).astype(bf16)

    core = dst // NPC
    blk = (dst % NPC) // 128
    dst_local = (dst % NPC) % 128

    grp = core * NB + blk
    NG = P * NB
    counts = np.bincount(grp, minlength=NG)
    T_B = int(np.ceil(counts.max() / 128))
    L = T_B * 128
    NT = NB * T_B

    order = np.argsort(grp, kind="stable")
    starts = np.zeros(NG + 1, np.int64)
    np.cumsum(counts, out=starts[1:])
    gs = grp[order]
    pos = np.arange(E) - starts[gs]
    slot = gs * L + pos

    idx_pad = np.zeros(NG * L, np.int32)
    dl_pad = np.full(NG * L, 255.0, np.float32)
    idx_pad[slot] = src[order].astype(np.int32)
    dl_pad[slot] = dst_local[order].astype(np.float32)

    idx_pad = idx_pad.reshape(P, NT * 128)
    dl_pad = dl_pad.reshape(P, NT * 128)

    # normdst per core: [1, NB*128] row
    nd = np.ones((P, NB * 128), np.float32)
    nd[:, :NPC] = norm_dst.reshape(P, NPC)

    iota = np.broadcast_to(np.arange(128, dtype=np.float32), (128, 128)).astype(bf16)
    wconv = np.ascontiguousarray(
        np.asarray(W_conv, np.float32).reshape(4, 128, H).transpose(1, 0, 2)
    ).reshape(128, 4 * H).astype(bf16)
    w2r = np.ascontiguousarray(
        np.asarray(W2, np.float32).reshape(2, 128, C).transpose(1, 0, 2)
    ).reshape(128, 2 * C).astype(bf16)

    in_maps = []
    for c in range(P):
        in_maps.append({
            "feat": feat_n,
            "idx32": np.ascontiguousarray(idx_pad[c].reshape(NT, 128).T),
            "dstloc": np.ascontiguousarray(
                dl_pad[c].reshape(NT, 128).T).astype(bf16),
            "normdst": np.ascontiguousarray(
                np.broadcast_to(nd[c], (128, NB * 128))),
            "iota": iota,
            "wconv": wconv,
            "w2": w2r,
            "ones1": np.ones((1, 128), np.float32).astype(bf16),
            "bconv": np.asarray(b_conv, np.float32).reshape(2, 128).T.copy(),
            "b2r": np.asarray(b2, np.float32).reshape(1, C).astype(bf16),
        })
    return T_B, in_maps


def kernel(features, W_conv, b_conv, W2, b2, src, dst):
    from concourse.bass_utils import run_bass_kernel_spmd

    T_B, in_maps = _prep(features, W_conv, b_conv, W2, b2, src, dst)
    key = (T_B, 0)
    if key not in _cache:
        _cache[key] = _build_program(T_B)
    nc = _cache[key]
    res = run_bass_kernel_spmd(nc, in_maps, core_ids=list(range(P)))
    out = np.concatenate([res.results[c]["out"] for c in range(P)], axis=0)
    return out.astype(np.float32)

